# revision 51
# baseline (speedup 1.0000x reference)
"""Multi-head attention layer (B=4, L=2048, D=1024, H=16) on 8 TRN2 NeuronCores.

Sharding: core c handles batch b = c//2 and heads [8*(c%2), 8*(c%2)+8) —
batch-parallel x tensor-parallel over heads.  Host sums the two partial
outputs per batch and adds bv@Wo + bo (bk drops exactly by softmax shift
invariance).

Per-core dataflow (all matmul inputs bf16, fp32 accumulation):
  qT/kT = Wq/Wk_slice as stationary against xT  -> [512, 2048] (e on partitions)
  v     = x @ Wv_slice (+ones col per head)      -> [2048, 8*65]
  scores: per (g, c2) one psum tile [128 s, (h2, 512 l)] written by an
  adjacent pair of K=64 matmuls targeting PE row groups 0-63/64-127 (they
  run concurrently).  A = exp on ACT, FD=1024 per instruction.
  AV: per (h2, j) part: 16-matmul accumulation (A^T chunk stationary,
  v_aug moving, N=65); ones col -> softmax denom in col 64; normalize via
  reciprocal+tensor_scalar (DVE); PE-transpose V into VT;
  out_partial = VT.T @ Wo_slice.

Scheduling: blocks pair-major (m outer, lt inner), one block = 8 score
groups g feeding 16 ACT exps of ~1114ns — the ACT engine is the roofline
(~285us busy) and everything else is paced to hide under it.  Per block:
score pairs ping-pong 2 psum tags (st0/st1); the previous block's AV runs
as 8 (h2,j) parts in g=1..4; qk-projection quanta stream as 4-matmul
halves woven between score groups (so the PE FIFO detour between exps
stays under ~1us); out-proj per-ls chains fill the g=0,5,6,7 slots of m=3
blocks.  Epilogue out-projections (ls 11-15) use vt chunks pre-transposed
during earlier blocks (chunk p is final once pair-p AV lands) and push
their DMAs on the scalar queue, which is idle after the last exp.

Input DMAs: host pre-arranges all tensors into [128, N] wide layouts so
each is one contiguous descriptor (dma_start issue costs ~0.6us); the two
HWDGE queues (sync + scalar) are loaded so the first projection's data
(wq + xT quarter 0) lands first.  Output partials are bf16 (halves the
out-DMA drain); the host sums them in fp32.
"""

import sys
from contextlib import ExitStack

for _p in ("/opt/trn_rl_repo", "/root/.axon_site/_ro/trn_rl_repo"):
    if _p not in sys.path:
        sys.path.append(_p)

import numpy as np
import ml_dtypes

import concourse.bass as bass
import concourse.mybir as mybir
import concourse.tile as tile
from concourse import bacc
from concourse.bass_utils import run_bass_kernel_spmd
from concourse.masks import make_identity

BF16 = mybir.dt.bfloat16
F32 = mybir.dt.float32
AF = mybir.ActivationFunctionType

B, L, D = 4, 2048, 1024
N_CORES = 8
DH = 512          # per-core head dims (8 heads x 64)
E = 64
SCALE = 0.125     # 1/sqrt(E)

KD = D // 128     # 8 contraction chunks for projections
NL = L // 512     # 4 l-tiles
NS = L // 128     # 16 s-chunks / l-subs


def build_attention_nc():
    nc = bacc.Bacc("TRN2", target_bir_lowering=False, debug=False)

    # Host pre-arranges everything into [128, N] wide-tile layouts so each
    # input is a single contiguous DMA (dma_start issue costs ~0.6us each).
    # xT: [p, 4096*q + 512*kd + c] = x.T[128*kd + p, 512*q + c]
    # wq/wk/wv: [p, 512*kd + c];  wo: [p, 1024*pq + c];  bq: [p, m]
    xT_d = nc.dram_tensor("xT", [128, KD * L], BF16, kind="ExternalInput").ap()
    wq_d = nc.dram_tensor("wq", [128, KD * DH], BF16, kind="ExternalInput").ap()
    wk_d = nc.dram_tensor("wk", [128, KD * DH], BF16, kind="ExternalInput").ap()
    wv_d = nc.dram_tensor("wv", [128, KD * DH], BF16, kind="ExternalInput").ap()
    wo_d = nc.dram_tensor("wo", [128, 4 * D], BF16, kind="ExternalInput").ap()
    bq_d = nc.dram_tensor("bq", [128, 4], F32, kind="ExternalInput").ap()
    out_d = nc.dram_tensor("out", [L, D], BF16, kind="ExternalOutput").ap()

    with tile.TileContext(nc) as tc, ExitStack() as ctx:
        const_pool = ctx.enter_context(tc.tile_pool(name="const", bufs=1))
        w_pool = ctx.enter_context(tc.tile_pool(name="w", bufs=1))
        qk_pool = ctx.enter_context(tc.tile_pool(name="qk", bufs=1))
        v_pool = ctx.enter_context(tc.tile_pool(name="v", bufs=1))
        at_pool = ctx.enter_context(tc.tile_pool(name="at", bufs=14))
        vs_pool = ctx.enter_context(tc.tile_pool(name="vs", bufs=1))
        vt_pool = ctx.enter_context(tc.tile_pool(name="vt", bufs=12))
        vtc_pool = ctx.enter_context(tc.tile_pool(name="vtc", bufs=1))
        rec_pool = ctx.enter_context(tc.tile_pool(name="rec", bufs=8))
        osb_pool = ctx.enter_context(tc.tile_pool(name="osb", bufs=2))

        st_ps = ctx.enter_context(tc.tile_pool(name="st_ps", bufs=1, space="PSUM"))
        av_ps = ctx.enter_context(tc.tile_pool(name="av_ps", bufs=1, space="PSUM"))
        tr_ps = ctx.enter_context(tc.tile_pool(name="tr_ps", bufs=1, space="PSUM"))
        out_ps = ctx.enter_context(tc.tile_pool(name="out_ps", bufs=1, space="PSUM"))
        phase1_ctx = ExitStack()
        xt_pool = phase1_ctx.enter_context(tc.tile_pool(name="xt", bufs=1))

        ident = const_pool.tile([128, 128], BF16, tag="ident", name="ident")
        make_identity(nc, ident[:])
        bq_sb = const_pool.tile([128, 4], F32, tag="bq", name="bq_sb")
        nc.sync.dma_start(bq_sb[:], bq_d[:, :])

        # Consolidated DMAs (each dma_start costs ~0.6us of issue time on the
        # Sync queue): one wide tile per tensor.  Order: wq/wk, xT by column
        # quarters, wv, wo — so the prologue projections start early.
        wq_sb = xt_pool.tile([128, KD * DH], BF16, tag="wq", name="wq_sb")
        wk_sb = xt_pool.tile([128, KD * DH], BF16, tag="wk", name="wk_sb")
        wv_sb = xt_pool.tile([128, KD * DH], BF16, tag="wv", name="wv_sb")
        xt_sb = xt_pool.tile([128, KD * L], BF16, tag="xt", name="xt_sb")
        wo_sb = w_pool.tile([128, 4 * D], BF16, tag="wo", name="wo_sb")
        # two hardware DMA queues in parallel (ACT is idle at prologue);
        # each queue moves ~180GB/s, so lead with what the first matmuls
        # need: scalar: xtq0, wv, xtq3, wo;  sync: bq, wq, wk, xtq1, xtq2
        def xt_dma(eng, q4):
            eng.dma_start(
                xt_sb[:, 4096 * q4 : 4096 * q4 + 4096],
                xT_d[:, 4096 * q4 : 4096 * q4 + 4096])
        xt_dma(nc.scalar, 0)
        nc.sync.dma_start(wq_sb[:], wq_d[:, :])
        nc.scalar.dma_start(wv_sb[:], wv_d[:, :])
        nc.sync.dma_start(wk_sb[:], wk_d[:, :])
        xt_dma(nc.scalar, 3)
        xt_dma(nc.sync, 1)
        nc.scalar.dma_start(wo_sb[:], wo_d[:, :])
        xt_dma(nc.sync, 2)


        qT = [qk_pool.tile([128, L], BF16, tag=f"qT{m}", name=f"qT{m}") for m in range(4)]
        kT = [qk_pool.tile([128, L], BF16, tag=f"kT{m}", name=f"kT{m}") for m in range(4)]

        PROJ_TILE = {"st0": (st_ps, [128, 512]), "st1": (st_ps, [128, 512]),
                     "outp": (out_ps, [128, 512]), "tr": (tr_ps, [128, 512]),
                     "av0": (av_ps, [128, 512]), "av1": (av_ps, [128, 512])}

        def emit_qk_proj(m, which, n, tag):
            # one psum group (8 matmuls) of the q or k projection
            pool, shape = PROJ_TILE[tag]
            ps = pool.tile(shape, F32, tag=tag, name="proj")
            w_sb = wq_sb if which == "q" else wk_sb
            for kd in range(KD):
                nc.tensor.matmul(
                    ps[:], w_sb[:, DH * kd + 128 * m : DH * kd + 128 * m + 128],
                    xt_sb[:, 4096 * n + 512 * kd : 4096 * n + 512 * kd + 512],
                    start=(kd == 0), stop=(kd == KD - 1))
            if which == "q":
                nc.vector.tensor_scalar_add(
                    qT[m][:, 512 * n : 512 * n + 512], ps[:], bq_sb[:, m : m + 1])
            else:
                nc.vector.tensor_copy(kT[m][:, 512 * n : 512 * n + 512], ps[:])

        v_aug = [None] * NS

        def emit_v_proj(s, tag):
            pool, shape = PROJ_TILE[tag]
            ps = pool.tile(shape, F32, tag=tag, name="proj")
            for kd in range(KD):
                nc.tensor.matmul(
                    ps[:], xt_sb[:, 4096 * (s // 4) + 512 * kd + 128 * (s % 4)
                           : 4096 * (s // 4) + 512 * kd + 128 * (s % 4) + 128],
                    wv_sb[:, DH * kd : DH * kd + DH],
                    start=(kd == 0), stop=(kd == KD - 1))
            t = v_pool.tile([128, 520], BF16, tag=f"v{s}", name=f"vaug{s}")
            t3 = t[:].rearrange("p (h e) -> p h e", h=8)
            nc.vector.tensor_copy(t3[:, :, 0:64], ps[:].rearrange("p (h e) -> p h e", h=8))
            nc.vector.memset(t3[:, :, 64:65], 1.0)
            v_aug[s] = t

        # ---- prologue: only what block (0,0)'s first scores need — the
        #      rest weaves into the blocks so the first exp runs ASAP ----
        emit_qk_proj(0, "q", 0, "st0")
        emit_qk_proj(0, "k", 0, "outp")
        emit_qk_proj(0, "k", 1, "tr")
        emit_qk_proj(0, "k", 2, "av0")

        # qk-projection quanta: block (m,lt) -> list of (pm, which, n),
        # emitted as 4-matmul halves woven between score pairs, each quantum
        # completing before its first use.
        quanta = {
            (0, 0): [(0, "k", 3), (0, "q", 1)],
            (0, 1): [(0, "q", 2)],
            (0, 2): [(0, "q", 3), (1, "k", 0), (1, "k", 1), (1, "k", 2)],
            (0, 3): [(1, "k", 3), (1, "q", 0), (1, "q", 1), (1, "q", 2)],
            (1, 0): [(1, "q", 3), (2, "k", 0), (2, "k", 1), (2, "k", 2)],
            (1, 1): [(2, "k", 3), (2, "q", 0), (2, "q", 1), (2, "q", 2)],
            (1, 2): [(2, "q", 3), (3, "k", 0), (3, "k", 1), (3, "k", 2)],
            (1, 3): [(3, "k", 3), (3, "q", 0), (3, "q", 1), (3, "q", 2)],
            (2, 0): [(3, "q", 3)],
        }

        # ---- attention blocks ----
        vstage = [vs_pool.tile([128, DH], BF16, tag=f"vs{ls}", name=f"vs{ls}") for ls in range(NS)]

        def emit_av_part(prev, part):
            # one (h2, j) slice of the AV sweep for block `prev`
            pm, plt, ats = prev
            h2, j = divmod(part, 4)
            h = 2 * pm + h2
            avp = av_ps.tile([128, 65], F32, tag=f"av{part % 2}", name="avp")
            for s in range(NS):
                nc.tensor.matmul(
                    avp[:], ats[s][:, 512 * h2 + 128 * j : 512 * h2 + 128 * j + 128],
                    v_aug[s][:, 65 * h : 65 * h + 65],
                    start=(s == 0), stop=(s == NS - 1))
            r = rec_pool.tile([128, 1], F32, tag="rec", name="rec")
            nc.vector.reciprocal(r[:], avp[:, 64:65])
            nc.vector.tensor_scalar_mul(
                vstage[4 * plt + j][:, 64 * h : 64 * h + 64], avp[:, 0:64], r[:])

        vt_cache = {}

        def emit_tr_p(ls, p, tag="tr", cache=False):
            # one PE transpose of a vstage 128-col chunk -> vt tile
            pool = PROJ_TILE[tag][0]
            tp = pool.tile([128, 128], BF16, tag=tag, name="trp")
            nc.tensor.transpose(tp[:], vstage[ls][:, 128 * p : 128 * p + 128], ident[:])
            if cache:
                vt = vtc_pool.tile([128, 128], BF16, tag=f"vtc{ls}_{p}", name="vtc")
                vt_cache[(ls, p)] = vt
            else:
                vt = vt_pool.tile([128, 128], BF16, tag="vt", name="vt")
            nc.vector.tensor_copy(vt[:], tp[:])
            return vt

        def emit_outproj_tr(ls, tr_tag="tr"):
            return [emit_tr_p(ls, p, tr_tag) for p in range(4)]

        def emit_outproj_mm(ls, vts, op_tag="outp", dma_eng=None, op_tag2=None):
            # 2 psum groups -> osb (bf16) -> DMA; with op_tag2 the two d2
            # groups use separate banks so neither waits the other's copy
            osb = osb_pool.tile([128, D], BF16, tag="osb", name="osb")
            for d2 in range(2):
                tag = op_tag if d2 == 0 or op_tag2 is None else op_tag2
                pool = PROJ_TILE[tag][0]
                op = pool.tile([128, 512], F32, tag=tag, name="outp")
                for p in range(4):
                    nc.tensor.matmul(
                        op[:], vts[p][:],
                        wo_sb[:, D * p + 512 * d2 : D * p + 512 * d2 + 512],
                        start=(p == 0), stop=(p == 3))
                nc.vector.tensor_copy(osb[:, 512 * d2 : 512 * d2 + 512], op[:])
            (dma_eng or nc.sync).dma_start(out_d[128 * ls : 128 * ls + 128, :], osb[:])

        def emit_outproj_ls(ls):
            emit_outproj_mm(ls, emit_outproj_tr(ls))

        outproj_q = []
        qk_state = {"cur": None, "ntag": 0}
        # pre-transpose schedule for the epilogue out-projections (ls 11-15):
        # chunk p of vstage[ls] is final once pair-p AV lands, so transpose it
        # during the pair-(p+1) blocks instead of on the critical tail
        pretr = {}
        for pm in range(1, 4):
            pretr[(pm, 0)] = [(12, pm - 1)]
            pretr[(pm, 1)] = [(13, pm - 1), (11, pm - 1)]
            pretr[(pm, 2)] = [(14, pm - 1)]
            pretr[(pm, 3)] = [(15, pm - 1)]

        def emit_qk_half(q):
            # one 4-matmul half of a qk-projection quantum; keeps the PE
            # FIFO detour between score pairs under ~1us
            if qk_state["cur"] is None:
                if not q:
                    return
                m_, w_, n_ = q.pop(0)
                tag = ("outp", "tr")[qk_state["ntag"] % 2]
                qk_state["ntag"] += 1
                pool, shape = PROJ_TILE[tag]
                ps = pool.tile(shape, F32, tag=tag, name="proj")
                kds = range(0, 4)
                qk_state["cur"] = (ps, m_, w_, n_)
            else:
                ps, m_, w_, n_ = qk_state["cur"]
                kds = range(4, KD)
            w_sb = wq_sb if w_ == "q" else wk_sb
            for kd in kds:
                nc.tensor.matmul(
                    ps[:], w_sb[:, DH * kd + 128 * m_ : DH * kd + 128 * m_ + 128],
                    xt_sb[:, 4096 * n_ + 512 * kd : 4096 * n_ + 512 * kd + 512],
                    start=(kd == 0), stop=(kd == KD - 1))
            if kds.stop == KD:
                if w_ == "q":
                    nc.vector.tensor_scalar_add(
                        qT[m_][:, 512 * n_ : 512 * n_ + 512], ps[:], bq_sb[:, m_ : m_ + 1])
                else:
                    nc.vector.tensor_copy(kT[m_][:, 512 * n_ : 512 * n_ + 512], ps[:])
                qk_state["cur"] = None

        def emit_block(m, lt, prev, outproj_new):
            # one (head-pair, l-tile) block: 8 score groups g; each g makes
            # one [128, 1024] psum tile per c2 (both h2 halves, adjacent
            # matmul pair -> concurrent PE row groups) and exps it.
            q = list(quanta.get((m, lt), []))
            ptq = list(pretr.get((m, lt), []))
            vq = (list(range(12)) if (m, lt) == (0, 0)
                  else list(range(12, NS)) if (m, lt) == (0, 1) else [])
            ats = [None] * NS
            for g in range(8):
                for c2 in range(2):
                    s = 2 * g + c2
                    stc = st_ps.tile([128, 1024], F32, tag=f"st{c2}", name=f"st{c2}")
                    for h2 in range(2):
                        p0 = 64 * h2
                        nc.tensor.matmul(
                            stc[:, 512 * h2 : 512 * h2 + 512],
                            kT[m][p0 : p0 + 64, 128 * s : 128 * s + 128],
                            qT[m][p0 : p0 + 64, 512 * lt : 512 * lt + 512],
                            start=True, stop=True)
                    at = at_pool.tile([128, 1024], BF16, tag=f"at{c2}", name=f"at{c2}")
                    nc.scalar.activation(at[:], stc[:], AF.Exp, scale=SCALE)
                    ats[s] = at
                if vq:
                    emit_v_proj(vq.pop(0), ["av0", "av1", "outp", "tr"][g % 4])
                    if vq and ((m, lt) == (0, 1) or g % 2 == 0):
                        emit_v_proj(vq.pop(0), ["av1", "outp", "tr", "av0"][g % 4])
                if 1 <= g <= 4 and prev is not None:
                    emit_av_part(prev, 2 * (g - 1))
                    emit_av_part(prev, 2 * (g - 1) + 1)
                if g in (3, 4) and ptq:
                    ls_, p_ = ptq.pop(0)
                    emit_tr_p(ls_, p_, tag="av0" if g == 3 else "av1", cache=True)
                if g == 5:
                    # this block's lt-1 out-proj becomes legal once the AV
                    # parts above (g=1..4) have filled vstage
                    outproj_q.extend(outproj_new)
                    outproj_new = []
                want_half = (q or qk_state["cur"] is not None) and (
                    g % 2 == 1 if (m, lt) == (0, 0) else not vq)
                if want_half:
                    emit_qk_half(q)
                elif not vq and g in (0, 5, 6, 7) and outproj_q:
                    emit_outproj_ls(outproj_q.pop(0))
            return ats

        prev = None
        for m in range(4):
            for lt in range(NL):
                new = [4 * (lt - 1) + i for i in range(4)] if (m == 3 and lt > 0) else []
                ats = emit_block(m, lt, prev, new)
                prev = (m, lt, ats)
        # epilogue: AV of the last block, then out-proj of ls 11..15 using the
        # pre-transposed p=0..2 chunks (only p=3 is fresh); output DMAs on the
        # now-idle scalar queue
        def outproj_cached(ls, tr_tag, op_tag, op_tag2=None):
            vts = [vt_cache[(ls, p)] for p in range(3)]
            vts.append(emit_tr_p(ls, 3, tr_tag))
            emit_outproj_mm(ls, vts, op_tag, dma_eng=nc.scalar, op_tag2=op_tag2)

        emit_av_part(prev, 0)
        emit_av_part(prev, 4)
        for ls in outproj_q:             # leftover from the block loop (ls 11)
            outproj_cached(ls, "tr", "outp", "tr")
        emit_av_part(prev, 1)
        emit_av_part(prev, 5)
        outproj_cached(12, "tr", "outp", "tr")
        emit_av_part(prev, 2)
        emit_av_part(prev, 6)
        emit_av_part(prev, 3)
        emit_av_part(prev, 7)
        outproj_cached(13, "av0", "av1", "av0")
        outproj_cached(14, "tr", "outp", "tr")
        outproj_cached(15, "av0", "av1", "av0")
        phase1_ctx.close()

    nc.compile()
    return nc


_NC_CACHE = []


def _make_in_maps(inputs):
    x = np.asarray(inputs["x"], dtype=np.float32)
    Wq = np.asarray(inputs["Wq"], dtype=np.float32)
    Wk = np.asarray(inputs["Wk"], dtype=np.float32)
    Wv = np.asarray(inputs["Wv"], dtype=np.float32)
    Wo = np.asarray(inputs["Wo"], dtype=np.float32)
    bq = np.asarray(inputs["bq"], dtype=np.float32)
    bf = ml_dtypes.bfloat16

    def wide_kd(w):  # [1024, C] -> [128, 8*C], kd-chunks side by side
        c = w.shape[1]
        return np.ascontiguousarray(
            w.reshape(KD, 128, c).transpose(1, 0, 2).reshape(128, KD * c))

    in_maps = []
    for c in range(N_CORES):
        b, hh = divmod(c, 2)
        sl = slice(DH * hh, DH * hh + DH)
        xT = x[b].T  # [1024, 2048]
        # [p, 4096*q + 512*kd + col] = xT[128*kd + p, 512*q + col]
        xT_w = xT.reshape(KD, 128, 4, 512).transpose(1, 2, 0, 3).reshape(128, KD * L)
        wo_w = Wo[sl, :].reshape(4, 128, D).transpose(1, 0, 2).reshape(128, 4 * D)
        in_maps.append({
            "xT": np.ascontiguousarray(xT_w).astype(bf),
            "wq": wide_kd(Wq[:, sl]).astype(bf),
            "wk": wide_kd(Wk[:, sl]).astype(bf),
            "wv": wide_kd(Wv[:, sl]).astype(bf),
            "wo": np.ascontiguousarray(wo_w).astype(bf),
            "bq": np.ascontiguousarray(bq[sl].reshape(4, 128).T).astype(np.float32),
        })
    return in_maps


def kernel(x, Wq, bq, Wk, bk, Wv, bv, Wo, bo):
    x = np.asarray(x, dtype=np.float32)
    Wq = np.asarray(Wq, dtype=np.float32)
    Wk = np.asarray(Wk, dtype=np.float32)
    Wv = np.asarray(Wv, dtype=np.float32)
    Wo = np.asarray(Wo, dtype=np.float32)
    bq = np.asarray(bq, dtype=np.float32)
    bv = np.asarray(bv, dtype=np.float32)
    bo = np.asarray(bo, dtype=np.float32)

    if not _NC_CACHE:
        _NC_CACHE.append(build_attention_nc())
    nc = _NC_CACHE[0]

    in_maps = _make_in_maps(dict(x=x, Wq=Wq, bq=bq, Wk=Wk, Wv=Wv, Wo=Wo))

    res = run_bass_kernel_spmd(nc, in_maps, list(range(N_CORES)))
    parts = [np.asarray(res.results[c]["out"], dtype=np.float32)
             for c in range(N_CORES)]
    out = np.stack([parts[2 * b] + parts[2 * b + 1] for b in range(B)])
    out += (bv @ Wo + bo)[None, None, :]
    return out.astype(np.float32)
